# revision 10
# baseline (speedup 1.0000x reference)
"""DirectedLowRankEdgeScorer TRN2 Bass kernel (8 NeuronCores, SPMD).

logits[b,l,e] = sum_r a[b,I[e],r] * gamma[l,r] * b[b,J[e],r]
  a = relu(H@W1s+b1s)@W2s+b2s,  b = relu(H@W1d+b1d)@W2d+b2d,  H = X[:,-1]

Per-core plan:
  1. MLP (true fp32 on PE) over this core's 1/8 node shard -> 256B records
     rec[n] = [a0 a1 b0 b1] (4x16 fp32), written to DRAM.
  2. AllGather record shards -> full record table in every core's HBM.
  3. Edge shard (E/8) gathered via dma_gather (4 SWDGE queues, 256B/desc,
     int16 indices; host pre-groups edges by (I>=32768, J>=32768) so each
     1024-gather uses a single table base).
  4. DVE: prod = aI * bJ (both batches, 32 wide).  PE: transpose 128x32 ->
     32x128, then block-diag gamma matmul in 3x-bf16 (hi/lo split, fp32
     accurate) -> psum [96, 512] = (4 subtiles x 2 batches x 12 layers).
  5. Stream [96, 512] tiles to DRAM; host un-permutes to (B, L, E).
"""

import sys
import types

import numpy as np
import ml_dtypes

import bass_rust
import concourse.bass as bass
import concourse.bacc as bacc
import concourse.mybir as mybir
from concourse.bass_utils import run_bass_kernel_spmd
from concourse.tile import TileContext
from concourse.vector_clock import ScopedClock
from concourse.masks import make_identity

BF16 = ml_dtypes.bfloat16

B, T, N, C = 2, 8, 50000, 64
HID, R, L, E = 128, 16, 12, 1600000
NCORES = 8
ESHARD = E // NCORES          # 200000 edges per core
NP = 6272                     # nodes per core shard (49*128)
NPAD = NP * NCORES            # 50176 padded node count
CHUNK = 32768                 # int16 index reach; table split at this row
TG = 2048                     # edges per tile-group (2x1024 gathers/side)
KG = 1024                     # indices per dma_gather


# ---------------------------------------------------------------- patches
def _patched_drain_and_barrier(self, tick_clock, wait_clock):
    nc = self.nc
    probe = nc.sync.drain()
    wait_clock.add_sem_waits(probe.ins, ScopedClock({None: tick_clock.global_clock}))
    si = probe.ins.sync_info
    waits = list(si.on_wait) if si is not None else []
    if len(waits) > 1:
        si.on_wait.clear()
        si.on_wait.append(waits[0])
        for w in waits[1:]:
            ni = nc.sync.drain().ins
            ni.sync_info = bass_rust.SyncInfo(on_wait=[w], on_update=[])
    nc.all_engine_barrier()
    assert self.sems is not None
    popped = nc._tile_sem_poison_stack.pop()
    assert popped is self._sem_poison
    nc.clear_and_free_semaphores(list(self.sems.allocated().values()))
    nc.all_engine_barrier()


TileContext._drain_and_barrier = _patched_drain_and_barrier

if "antenv.axon_hooks" not in sys.modules:
    _mod = types.ModuleType("antenv.axon_hooks")
    _state = {"hook": None}
    _mod.set_axon_ntff_profile_hook = lambda h: _state.__setitem__("hook", h)
    _mod.get_axon_ntff_profile_hook = lambda: _state["hook"]
    sys.modules["antenv.axon_hooks"] = _mod
    try:
        import antenv

        antenv.axon_hooks = _mod
    except Exception:
        pass
    try:
        from trn_agent_boot.trn_boot import _ntff_profile_via_ctypes

        _hook = _ntff_profile_via_ctypes("/opt/axon/libaxon_pjrt.so")
        if _hook is not None:
            _mod.set_axon_ntff_profile_hook(_hook)
    except Exception:
        pass


# ---------------------------------------------------------------- device
_PROGRAM_CACHE = {}


def build_program(tiles_per_group):
    """tiles_per_group: 4 ints, 2048-edge tile-group count per (cI, cJ) group."""
    nT2 = sum(tiles_per_group)
    nT = 2 * nT2
    f32, bf16, i16 = mybir.dt.float32, mybir.dt.bfloat16, mybir.dt.int16

    nc = bacc.Bacc("TRN2", target_bir_lowering=False, num_swdge_queues=4)

    HT = nc.declare_dram_parameter("HT", [B, C, NP], f32, isOutput=False)
    W1 = nc.declare_dram_parameter("W1", [2, C, HID], f32, isOutput=False)
    B1 = nc.declare_dram_parameter("B1", [2, HID, 1], f32, isOutput=False)
    W2 = nc.declare_dram_parameter("W2", [2, HID, R], f32, isOutput=False)
    B2 = nc.declare_dram_parameter("B2", [2, 128, R], f32, isOutput=False)
    GBD = nc.declare_dram_parameter("GBD", [2, 128, 96], bf16, isOutput=False)
    IDXA = nc.declare_dram_parameter("IDXA", [nT, 128, KG // 16], i16, isOutput=False)
    IDXB = nc.declare_dram_parameter("IDXB", [nT, 128, KG // 16], i16, isOutput=False)
    OUT = nc.declare_dram_parameter("OUT", [96, nT2 * 512], f32, isOutput=True)

    rec_shard = nc.dram_tensor("rec_shard", [NP, 64], f32)
    idxa_all = nc.alloc_sbuf_tensor("idxa_all", [128, nT, KG // 16], mybir.dt.int16)
    idxb_all = nc.alloc_sbuf_tensor("idxb_all", [128, nT, KG // 16], mybir.dt.int16)
    rec_full = nc.dram_tensor("rec_full", [NPAD, 64], f32, addr_space="Shared")

    # --- stage A: MLP over node shard -> records
    with TileContext(nc) as tc:
        with (
            tc.tile_pool(name="const", bufs=1) as constp,
            tc.tile_pool(name="h1p", bufs=3) as h1p,
            tc.tile_pool(name="recp", bufs=3) as recp,
            tc.tile_pool(name="ps1", bufs=2, space="PSUM") as ps1,
            tc.tile_pool(name="ps2", bufs=4, space="PSUM") as ps2,
        ):
            w1_s = constp.tile([C, 2, HID], f32)
            nc.sync.dma_start(w1_s[:], W1[:].rearrange("t c h -> c t h"))
            b1_s = constp.tile([HID, 2, 1], f32)
            nc.sync.dma_start(b1_s[:], B1[:].rearrange("t h o -> h t o"))
            w2_s = constp.tile([HID, 2, R], f32)
            nc.sync.dma_start(w2_s[:], W2[:].rearrange("t h r -> h t r"))
            b2_s = constp.tile([128, 2, R], f32)
            nc.sync.dma_start(b2_s[:], B2[:].rearrange("t p r -> p t r"))
            ht_s = constp.tile([C, B, NP], f32)
            nc.sync.dma_start(ht_s[:], HT[:].rearrange("b c n -> c b n"))
            nc.sync.dma_start(idxa_all[:], IDXA[:].rearrange("t p x -> p t x"))
            nc.sync.dma_start(idxb_all[:], IDXB[:].rearrange("t p x -> p t x"))

            n_chunks = NP // 512 + (1 if NP % 512 else 0)
            for ch in range(n_chunks):
                n0 = ch * 512
                csz = min(512, NP - n0)
                nsub = csz // 128
                h1s = []
                for t in range(2):
                    for b in range(B):
                        p1 = ps1.tile([HID, csz], f32, tag="p1")
                        nc.tensor.matmul(
                            p1[:],
                            w1_s[:, t, :],
                            ht_s[:, b, n0:n0 + csz],
                        )
                        h1 = h1p.tile([HID, csz], f32, tag=f"h1_{t}_{b}")
                        nc.scalar.activation(
                            h1[:], p1[:],
                            mybir.ActivationFunctionType.Relu,
                            bias=b1_s[:, t, :], scale=1.0,
                        )
                        h1s.append((t, b, h1))
                for s in range(nsub):
                    rec = recp.tile([128, 64], f32, tag="rec")
                    for (t, b, h1) in h1s:
                        p2 = ps2.tile([128, R], f32, tag="p2")
                        nc.tensor.matmul(
                            p2[:],
                            h1[:, s * 128:(s + 1) * 128],
                            w2_s[:, t, :],
                        )
                        co = 32 * t + 16 * b
                        nc.vector.tensor_add(
                            rec[:, co:co + 16], p2[:], b2_s[:, t, :]
                        )
                    nc.sync.dma_start(rec_shard[n0 + s * 128: n0 + (s + 1) * 128, :], rec[:])

    # --- collective: AllGather record shards
    cc_sem = nc.alloc_semaphore("cc_sem")
    with nc.Block() as blk:
        @blk.gpsimd
        def _(gpsimd):
            gpsimd.collective_compute(
                "AllGather",
                mybir.AluOpType.bypass,
                replica_groups=[list(range(NCORES))],
                ins=[rec_shard[:]],
                outs=[rec_full[:]],
            ).then_inc(cc_sem, 1)
            gpsimd.wait_ge(cc_sem, 1)

    # --- stage B: gather + score
    group_of_tile = []
    for g in range(4):
        group_of_tile += [g] * tiles_per_group[g]

    with TileContext(nc) as tc:
        with (
            tc.tile_pool(name="bconst", bufs=1) as constp,
            tc.tile_pool(name="gp", bufs=4) as gp,
            tc.tile_pool(name="prodp", bufs=3) as prodp,
            tc.tile_pool(name="ctp", bufs=3) as ctp,
            tc.tile_pool(name="outp", bufs=3) as outp,
            tc.tile_pool(name="psT", bufs=3, space="PSUM") as psT,
            tc.tile_pool(name="psL", bufs=3, space="PSUM") as psL,
        ):
            gbd_s = constp.tile([128, 2, 96], bf16)
            nc.sync.dma_start(gbd_s[:], GBD[:].rearrange("t p m -> p t m"))
            ident = constp.tile([128, 128], f32)
            make_identity(nc, ident[:])

            for Ti in range(nT2):
                g = group_of_tile[Ti]
                cI, cJ = g >> 1, g & 1
                baseA = CHUNK * cI
                baseB = CHUNK * cJ

                gA = gp.tile([128, 16, 64], f32, tag="gA")
                gB = gp.tile([128, 16, 64], f32, tag="gB")
                for h in range(2):
                    nc.gpsimd.dma_gather(
                        gA[:, 8 * h:8 * (h + 1), :],
                        rec_full[baseA:, :],
                        idxa_all[:, 2 * Ti + h, :],
                        num_idxs=KG, num_idxs_reg=KG, elem_size=64,
                        single_packet=False,
                        queue_num=(2 * Ti + h) % 4,
                    )
                    nc.gpsimd.dma_gather(
                        gB[:, 8 * h:8 * (h + 1), :],
                        rec_full[baseB:, :],
                        idxb_all[:, 2 * Ti + h, :],
                        num_idxs=KG, num_idxs_reg=KG, elem_size=64,
                        single_packet=False,
                        queue_num=(2 * Ti + h + 2) % 4,
                    )

                prod = prodp.tile([128, 16, 32], f32, tag="prod")
                nc.vector.tensor_mul(prod[:], gA[:, :, 0:32], gB[:, :, 32:64])

                pT = psT.tile([128, 512], f32, tag="pT")
                for gg in range(4):
                    nc.tensor.transpose(
                        pT[:, 128 * gg:128 * (gg + 1)],
                        prod[:, 4 * gg:4 * (gg + 1), :],
                        ident[:],
                    )
                cHi = ctp.tile([128, 512], bf16, tag="cHi")
                nc.scalar.copy(cHi[:], pT[:])
                cLo = ctp.tile([128, 512], bf16, tag="cLo")
                nc.vector.tensor_sub(cLo[:], pT[:], cHi[:])

                pL = psL.tile([96, 512], f32, tag="pL")
                nc.tensor.matmul(pL[:], gbd_s[:, 0, :], cHi[:], start=True, stop=False)
                nc.tensor.matmul(pL[:], gbd_s[:, 0, :], cLo[:], start=False, stop=False)
                nc.tensor.matmul(pL[:], gbd_s[:, 1, :], cHi[:], start=False, stop=True)

                outS = outp.tile([96, 512], f32, tag="outS")
                nc.scalar.copy(outS[:], pL[:])
                nc.sync.dma_start(OUT[:, 512 * Ti:512 * (Ti + 1)], outS[:])

    nc.finalize()
    return nc


# ---------------------------------------------------------------- host
def _wrap_idx(flat_idx):
    """[KG] int16 -> [128, KG//16] wrapped-16, replicated x8."""
    w = flat_idx.reshape(KG // 16, 16).T
    return np.tile(w, (8, 1))


def kernel(X, edge_index, W1s, b1s, W2s, b2s, W1d, b1d, W2d, b2d, gamma):
    X = np.asarray(X)
    edge_index = np.asarray(edge_index)
    H = np.ascontiguousarray(X[:, -1]).astype(np.float32)          # (B, N, C)
    Hp = np.zeros((B, NPAD, C), np.float32)
    Hp[:, :N] = H

    I = edge_index[0].astype(np.int64)
    J = edge_index[1].astype(np.int64)

    # per-core edge groups
    core_data = []
    counts = np.zeros((NCORES, 4), np.int64)
    for c in range(NCORES):
        Ic = I[c * ESHARD:(c + 1) * ESHARD]
        Jc = J[c * ESHARD:(c + 1) * ESHARD]
        gk = (Ic >= CHUNK).astype(np.int64) * 2 + (Jc >= CHUNK).astype(np.int64)
        order = np.argsort(gk, kind="stable")
        core_data.append((Ic[order], Jc[order], gk[order], order))
        for g in range(4):
            counts[c, g] = int((gk == g).sum())

    tiles_per_group = tuple(
        int(-(-counts[:, g].max() // TG)) for g in range(4)
    )
    nT2 = sum(tiles_per_group)
    nT = 2 * nT2
    Epad = nT2 * TG

    key = tiles_per_group
    if key not in _PROGRAM_CACHE:
        _PROGRAM_CACHE[key] = build_program(tiles_per_group)
    nc = _PROGRAM_CACHE[key]

    # group -> padded tile offsets
    goff = np.zeros(5, np.int64)
    for g in range(4):
        goff[g + 1] = goff[g] + tiles_per_group[g]

    # shared weight tensors
    W1 = np.stack([W1s, W1d]).astype(np.float32)                  # (2, C, HID)
    B1 = np.stack([b1s, b1d]).astype(np.float32)[:, :, None]      # (2, HID, 1)
    W2 = np.stack([W2s, W2d]).astype(np.float32)                  # (2, HID, R)
    B2 = np.stack(
        [np.tile(b2s[None, :], (128, 1)), np.tile(b2d[None, :], (128, 1))]
    ).astype(np.float32)                                          # (2, 128, R)

    gbd = np.zeros((128, 96), np.float32)
    gT = np.asarray(gamma, np.float32).T                          # (R, L)
    for g in range(4):
        for b in range(B):
            gbd[32 * g + 16 * b:32 * g + 16 * b + 16,
                24 * g + 12 * b:24 * g + 12 * b + 12] = gT
    gbd_hi = gbd.astype(BF16)
    gbd_lo = (gbd - gbd_hi.astype(np.float32)).astype(BF16)
    GBD = np.stack([gbd_hi, gbd_lo])                              # (2, 128, 96)

    in_maps = []
    unperm = []  # per core: padded position -> original in-shard position (or -1)
    for c in range(NCORES):
        Ic, Jc, gk, order = core_data[c]
        idxA = np.zeros((nT, 128, KG // 16), np.int16)
        idxB = np.zeros((nT, 128, KG // 16), np.int16)
        pad_pos = np.full(Epad, -1, np.int64)
        rp = 0  # read ptr into sorted arrays
        for g in range(4):
            cnt = int(counts[c, g])
            base_t = int(goff[g])
            Ig = Ic[rp:rp + cnt] - CHUNK * (g >> 1)
            Jg = Jc[rp:rp + cnt] - CHUNK * (g & 1)
            og = order[rp:rp + cnt]
            rp += cnt
            ntile_g = tiles_per_group[g]
            Ipad = np.zeros(ntile_g * TG, np.int64)
            Jpad = np.zeros(ntile_g * TG, np.int64)
            Ipad[:cnt] = Ig
            Jpad[:cnt] = Jg
            pad_pos[base_t * TG: base_t * TG + cnt] = og
            for t in range(ntile_g):
                for h in range(2):
                    sl = slice(t * TG + h * KG, t * TG + (h + 1) * KG)
                    idxA[2 * (base_t + t) + h] = _wrap_idx(Ipad[sl].astype(np.int16))
                    idxB[2 * (base_t + t) + h] = _wrap_idx(Jpad[sl].astype(np.int16))
        unperm.append(pad_pos)

        HTs = np.ascontiguousarray(
            Hp[:, c * NP:(c + 1) * NP, :].transpose(0, 2, 1)
        )                                                          # (B, C, NP)
        in_maps.append({
            "HT": HTs, "W1": W1, "B1": B1, "W2": W2, "B2": B2,
            "GBD": GBD, "IDXA": idxA, "IDXB": idxB,
        })

    import os
    import tempfile
    trace = bool(os.environ.get("BASS_KERNEL_TRACE"))
    tdir = None
    if trace:
        base = "/root/problem/work"
        tdir = tempfile.mkdtemp(prefix="ktrace_", dir=base if os.path.isdir(base) else None)
    res = run_bass_kernel_spmd(
        nc, in_maps, list(range(NCORES)), trace=trace, tmpdir=tdir,
    )
    if trace:
        kernel.last_trace_dir = tdir
    if trace:
        kernel.last_exec_time_ns = res.exec_time_ns

    logits = np.empty((B, L, E), np.float32)
    for c in range(NCORES):
        dev = res.results[c]["OUT"]                                # (96, nT2*512)
        # partition p = 24*gg + 12*b + l ; col = 512*T + j ;
        # padded pos = 2048*T + 512*gg + j
        # partition p = 24*k + 12*b + l ; col = 512*T + 128*g + e ;
        # padded pos = 2048*T + 512*g + 128*k + e
        dv = dev.reshape(4, 2, L, nT2, 4, 128)                     # (k, b, l, T, g, e)
        dv = dv.transpose(1, 2, 3, 4, 0, 5).reshape(B, L, Epad)    # (b, l, padded)
        pad_pos = unperm[c]
        valid = pad_pos >= 0
        logits[:, :, c * ESHARD + pad_pos[valid]] = dv[:, :, valid]
    return logits


# revision 13
# speedup vs baseline: 1.0220x; 1.0220x over previous
"""DirectedLowRankEdgeScorer TRN2 Bass kernel (8 NeuronCores, SPMD).

logits[b,l,e] = sum_r a[b,I[e],r] * gamma[l,r] * b[b,J[e],r]
  a = relu(H@W1s+b1s)@W2s+b2s,  b = relu(H@W1d+b1d)@W2d+b2d,  H = X[:,-1]

Per-core plan:
  1. MLP (true fp32 on PE) over this core's 1/8 node shard -> 256B records
     rec[n] = [a0 a1 b0 b1] (4x16 fp32), written to DRAM.
  2. AllGather record shards -> full record table in every core's HBM.
  3. Edge shard (E/8) gathered via dma_gather (4 SWDGE queues, 256B/desc,
     int16 indices; host pre-groups edges by (I>=32768, J>=32768) so each
     1024-gather uses a single table base).
  4. DVE: prod = aI * bJ (both batches, 32 wide).  PE: transpose 128x32 ->
     32x128, then block-diag gamma matmul in 3x-bf16 (hi/lo split, fp32
     accurate) -> psum [96, 512] = (4 subtiles x 2 batches x 12 layers).
  5. Stream [96, 512] tiles to DRAM; host un-permutes to (B, L, E).
"""

import sys
import types

import numpy as np
import ml_dtypes

import bass_rust
import concourse.bass as bass
import concourse.bacc as bacc
import concourse.mybir as mybir
from concourse.bass_utils import run_bass_kernel_spmd
from concourse.tile import TileContext
from concourse.vector_clock import ScopedClock
from concourse.masks import make_identity

BF16 = ml_dtypes.bfloat16

B, T, N, C = 2, 8, 50000, 64
HID, R, L, E = 128, 16, 12, 1600000
NCORES = 8
ESHARD = E // NCORES          # 200000 edges per core
NP = 6272                     # nodes per core shard (49*128)
NPAD = NP * NCORES            # 50176 padded node count
CHUNK = 32768                 # int16 index reach; table split at this row
TG = 2048                     # edges per tile-group (2x1024 gathers/side)
KG = 1024                     # indices per dma_gather


# ---------------------------------------------------------------- patches
def _patched_drain_and_barrier(self, tick_clock, wait_clock):
    nc = self.nc
    probe = nc.sync.drain()
    wait_clock.add_sem_waits(probe.ins, ScopedClock({None: tick_clock.global_clock}))
    si = probe.ins.sync_info
    waits = list(si.on_wait) if si is not None else []
    if len(waits) > 1:
        si.on_wait.clear()
        si.on_wait.append(waits[0])
        for w in waits[1:]:
            ni = nc.sync.drain().ins
            ni.sync_info = bass_rust.SyncInfo(on_wait=[w], on_update=[])
    nc.all_engine_barrier()
    assert self.sems is not None
    popped = nc._tile_sem_poison_stack.pop()
    assert popped is self._sem_poison
    nc.clear_and_free_semaphores(list(self.sems.allocated().values()))
    nc.all_engine_barrier()


TileContext._drain_and_barrier = _patched_drain_and_barrier

if "antenv.axon_hooks" not in sys.modules:
    _mod = types.ModuleType("antenv.axon_hooks")
    _state = {"hook": None}
    _mod.set_axon_ntff_profile_hook = lambda h: _state.__setitem__("hook", h)
    _mod.get_axon_ntff_profile_hook = lambda: _state["hook"]
    sys.modules["antenv.axon_hooks"] = _mod
    try:
        import antenv

        antenv.axon_hooks = _mod
    except Exception:
        pass
    try:
        from trn_agent_boot.trn_boot import _ntff_profile_via_ctypes

        _hook = _ntff_profile_via_ctypes("/opt/axon/libaxon_pjrt.so")
        if _hook is not None:
            _mod.set_axon_ntff_profile_hook(_hook)
    except Exception:
        pass


# ---------------------------------------------------------------- device
_PROGRAM_CACHE = {}


def build_program(tiles_per_group):
    """tiles_per_group: 4 ints, 2048-edge tile-group count per (cI, cJ) group."""
    nT2 = sum(tiles_per_group)
    nT = 2 * nT2
    f32, bf16, i16 = mybir.dt.float32, mybir.dt.bfloat16, mybir.dt.int16

    nc = bacc.Bacc("TRN2", target_bir_lowering=False, num_swdge_queues=4)

    HT = nc.declare_dram_parameter("HT", [B, C, NP], f32, isOutput=False)
    W1 = nc.declare_dram_parameter("W1", [2, C, HID], f32, isOutput=False)
    B1 = nc.declare_dram_parameter("B1", [2, HID, 1], f32, isOutput=False)
    W2 = nc.declare_dram_parameter("W2", [2, HID, R], f32, isOutput=False)
    B2 = nc.declare_dram_parameter("B2", [2, 128, R], f32, isOutput=False)
    GBD = nc.declare_dram_parameter("GBD", [2, 128, 96], bf16, isOutput=False)
    IDXA = nc.declare_dram_parameter("IDXA", [nT, 128, KG // 16], i16, isOutput=False)
    IDXB = nc.declare_dram_parameter("IDXB", [nT, 128, KG // 16], i16, isOutput=False)
    OUT = nc.declare_dram_parameter("OUT", [96, nT2 * 512], f32, isOutput=True)

    rec_shard = nc.dram_tensor("rec_shard", [NP, 64], f32)
    idxa_all = nc.alloc_sbuf_tensor("idxa_all", [128, nT, KG // 16], mybir.dt.int16)
    idxb_all = nc.alloc_sbuf_tensor("idxb_all", [128, nT, KG // 16], mybir.dt.int16)
    rec_full = nc.dram_tensor("rec_full", [NPAD, 64], f32, addr_space="Shared")

    # --- stage A: MLP over node shard -> records
    with TileContext(nc) as tc:
        with (
            tc.tile_pool(name="const", bufs=1) as constp,
            tc.tile_pool(name="h1p", bufs=1) as h1p,
            tc.tile_pool(name="recp", bufs=3) as recp,
            tc.tile_pool(name="ps1", bufs=2, space="PSUM") as ps1,
            tc.tile_pool(name="ps2", bufs=4, space="PSUM") as ps2,
        ):
            w1_s = constp.tile([C, 2, HID], f32)
            nc.sync.dma_start(w1_s[:], W1[:].rearrange("t c h -> c t h"))
            b1_s = constp.tile([HID, 2, 1], f32)
            nc.sync.dma_start(b1_s[:], B1[:].rearrange("t h o -> h t o"))
            w2_s = constp.tile([HID, 2, R], f32)
            nc.sync.dma_start(w2_s[:], W2[:].rearrange("t h r -> h t r"))
            b2_s = constp.tile([128, 2, R], f32)
            nc.sync.dma_start(b2_s[:], B2[:].rearrange("t p r -> p t r"))
            ht_s = constp.tile([C, B, NP], f32)
            nc.sync.dma_start(ht_s[:], HT[:].rearrange("b c n -> c b n"))
            nc.sync.dma_start(idxa_all[:], IDXA[:].rearrange("t p x -> p t x"))
            nc.sync.dma_start(idxb_all[:], IDXB[:].rearrange("t p x -> p t x"))

            # two half-shard passes; within each: all L1 back-to-back (PE
            # stays ramped), then all L2.  3200 + 3072 = NP.
            for (p0, psz) in ((0, 3200), (3200, 3072)):
                h1t = {}
                for t in range(2):
                    for b in range(B):
                        h1x = h1p.tile([HID, 3200], f32, tag=f"h1_{t}_{b}")
                        h1t[(t, b)] = h1x
                for n0 in range(0, psz, 512):
                    csz = min(512, psz - n0)
                    for t in range(2):
                        for b in range(B):
                            p1 = ps1.tile([HID, csz], f32, tag="p1")
                            nc.tensor.matmul(
                                p1[:],
                                w1_s[:, t, :],
                                ht_s[:, b, p0 + n0:p0 + n0 + csz],
                            )
                            nc.scalar.activation(
                                h1t[(t, b)][:, n0:n0 + csz], p1[:],
                                mybir.ActivationFunctionType.Relu,
                                bias=b1_s[:, t, :], scale=1.0,
                            )
                for s in range(psz // 128):
                    rec = recp.tile([128, 64], f32, tag="rec")
                    for t in range(2):
                        for b in range(B):
                            p2 = ps2.tile([128, R], f32, tag="p2")
                            nc.tensor.matmul(
                                p2[:],
                                h1t[(t, b)][:, s * 128:(s + 1) * 128],
                                w2_s[:, t, :],
                            )
                            co = 32 * t + 16 * b
                            nc.vector.tensor_add(
                                rec[:, co:co + 16], p2[:], b2_s[:, t, :]
                            )
                    n0 = p0 + s * 128
                    nc.sync.dma_start(rec_shard[n0:n0 + 128, :], rec[:])

    # --- collective: AllGather record shards
    cc_sem = nc.alloc_semaphore("cc_sem")
    with nc.Block() as blk:
        @blk.gpsimd
        def _(gpsimd):
            gpsimd.collective_compute(
                "AllGather",
                mybir.AluOpType.bypass,
                replica_groups=[list(range(NCORES))],
                ins=[rec_shard[:]],
                outs=[rec_full[:]],
            ).then_inc(cc_sem, 1)
            gpsimd.wait_ge(cc_sem, 1)

    # --- stage B: gather + score
    group_of_tile = []
    for g in range(4):
        group_of_tile += [g] * tiles_per_group[g]

    with TileContext(nc) as tc:
        with (
            tc.tile_pool(name="bconst", bufs=1) as constp,
            tc.tile_pool(name="gp", bufs=3) as gp,
            tc.tile_pool(name="prodp", bufs=3) as prodp,
            tc.tile_pool(name="ctp", bufs=3) as ctp,
            tc.tile_pool(name="outp", bufs=3) as outp,
            tc.tile_pool(name="psT", bufs=3, space="PSUM") as psT,
            tc.tile_pool(name="psL", bufs=3, space="PSUM") as psL,
        ):
            gbd_s = constp.tile([128, 2, 96], bf16)
            nc.sync.dma_start(gbd_s[:], GBD[:].rearrange("t p m -> p t m"))
            ident = constp.tile([128, 128], f32)
            make_identity(nc, ident[:])

            for Ti in range(nT2):
                g = group_of_tile[Ti]
                cI, cJ = g >> 1, g & 1
                baseA = CHUNK * cI
                baseB = CHUNK * cJ

                prods = []
                for h in range(2):
                    gA = gp.tile([128, 8, 64], f32, tag=f"gA{h}")
                    gB = gp.tile([128, 8, 64], f32, tag=f"gB{h}")
                    nc.gpsimd.dma_gather(
                        gA[:],
                        rec_full[baseA:, :],
                        idxa_all[:, 2 * Ti + h, :],
                        num_idxs=KG, num_idxs_reg=KG, elem_size=64,
                        single_packet=False,
                        queue_num=(2 * Ti + h) % 4,
                    )
                    nc.gpsimd.dma_gather(
                        gB[:],
                        rec_full[baseB:, :],
                        idxb_all[:, 2 * Ti + h, :],
                        num_idxs=KG, num_idxs_reg=KG, elem_size=64,
                        single_packet=False,
                        queue_num=(2 * Ti + h + 2) % 4,
                    )
                    prodh = prodp.tile([128, 8, 32], f32, tag=f"prod{h}")
                    nc.vector.tensor_mul(prodh[:], gA[:, :, 0:32], gB[:, :, 32:64])
                    prods.append(prodh)

                pT = psT.tile([128, 512], f32, tag="pT")
                for gg in range(4):
                    nc.tensor.transpose(
                        pT[:, 128 * gg:128 * (gg + 1)],
                        prods[gg // 2][:, 4 * (gg % 2):4 * (gg % 2 + 1), :],
                        ident[:],
                    )
                cHi = ctp.tile([128, 512], bf16, tag="cHi")
                nc.scalar.copy(cHi[:], pT[:])
                cLo = ctp.tile([128, 512], bf16, tag="cLo")
                nc.vector.tensor_sub(cLo[:], pT[:], cHi[:])

                pL = psL.tile([96, 512], f32, tag="pL")
                nc.tensor.matmul(pL[:], gbd_s[:, 0, :], cHi[:], start=True, stop=False)
                nc.tensor.matmul(pL[:], gbd_s[:, 0, :], cLo[:], start=False, stop=False)
                nc.tensor.matmul(pL[:], gbd_s[:, 1, :], cHi[:], start=False, stop=True)

                outS = outp.tile([96, 512], f32, tag="outS")
                nc.scalar.copy(outS[:], pL[:])
                nc.sync.dma_start(OUT[:, 512 * Ti:512 * (Ti + 1)], outS[:])

    nc.finalize()
    return nc


# ---------------------------------------------------------------- host
def _wrap_idx(flat_idx):
    """[KG] int16 -> [128, KG//16] wrapped-16, replicated x8."""
    w = flat_idx.reshape(KG // 16, 16).T
    return np.tile(w, (8, 1))


def kernel(X, edge_index, W1s, b1s, W2s, b2s, W1d, b1d, W2d, b2d, gamma):
    X = np.asarray(X)
    edge_index = np.asarray(edge_index)
    H = np.ascontiguousarray(X[:, -1]).astype(np.float32)          # (B, N, C)
    Hp = np.zeros((B, NPAD, C), np.float32)
    Hp[:, :N] = H

    I = edge_index[0].astype(np.int64)
    J = edge_index[1].astype(np.int64)

    # per-core edge groups
    core_data = []
    counts = np.zeros((NCORES, 4), np.int64)
    for c in range(NCORES):
        Ic = I[c * ESHARD:(c + 1) * ESHARD]
        Jc = J[c * ESHARD:(c + 1) * ESHARD]
        gk = (Ic >= CHUNK).astype(np.int64) * 2 + (Jc >= CHUNK).astype(np.int64)
        order = np.argsort(gk, kind="stable")
        core_data.append((Ic[order], Jc[order], gk[order], order))
        for g in range(4):
            counts[c, g] = int((gk == g).sum())

    tiles_per_group = tuple(
        int(-(-counts[:, g].max() // TG)) for g in range(4)
    )
    nT2 = sum(tiles_per_group)
    nT = 2 * nT2
    Epad = nT2 * TG

    key = tiles_per_group
    if key not in _PROGRAM_CACHE:
        _PROGRAM_CACHE[key] = build_program(tiles_per_group)
    nc = _PROGRAM_CACHE[key]

    # group -> padded tile offsets
    goff = np.zeros(5, np.int64)
    for g in range(4):
        goff[g + 1] = goff[g] + tiles_per_group[g]

    # shared weight tensors
    W1 = np.stack([W1s, W1d]).astype(np.float32)                  # (2, C, HID)
    B1 = np.stack([b1s, b1d]).astype(np.float32)[:, :, None]      # (2, HID, 1)
    W2 = np.stack([W2s, W2d]).astype(np.float32)                  # (2, HID, R)
    B2 = np.stack(
        [np.tile(b2s[None, :], (128, 1)), np.tile(b2d[None, :], (128, 1))]
    ).astype(np.float32)                                          # (2, 128, R)

    gbd = np.zeros((128, 96), np.float32)
    gT = np.asarray(gamma, np.float32).T                          # (R, L)
    for g in range(4):
        for b in range(B):
            gbd[32 * g + 16 * b:32 * g + 16 * b + 16,
                24 * g + 12 * b:24 * g + 12 * b + 12] = gT
    gbd_hi = gbd.astype(BF16)
    gbd_lo = (gbd - gbd_hi.astype(np.float32)).astype(BF16)
    GBD = np.stack([gbd_hi, gbd_lo])                              # (2, 128, 96)

    in_maps = []
    unperm = []  # per core: padded position -> original in-shard position (or -1)
    for c in range(NCORES):
        Ic, Jc, gk, order = core_data[c]
        idxA = np.zeros((nT, 128, KG // 16), np.int16)
        idxB = np.zeros((nT, 128, KG // 16), np.int16)
        pad_pos = np.full(Epad, -1, np.int64)
        rp = 0  # read ptr into sorted arrays
        for g in range(4):
            cnt = int(counts[c, g])
            base_t = int(goff[g])
            Ig = Ic[rp:rp + cnt] - CHUNK * (g >> 1)
            Jg = Jc[rp:rp + cnt] - CHUNK * (g & 1)
            og = order[rp:rp + cnt]
            rp += cnt
            ntile_g = tiles_per_group[g]
            Ipad = np.zeros(ntile_g * TG, np.int64)
            Jpad = np.zeros(ntile_g * TG, np.int64)
            Ipad[:cnt] = Ig
            Jpad[:cnt] = Jg
            pad_pos[base_t * TG: base_t * TG + cnt] = og
            for t in range(ntile_g):
                for h in range(2):
                    sl = slice(t * TG + h * KG, t * TG + (h + 1) * KG)
                    idxA[2 * (base_t + t) + h] = _wrap_idx(Ipad[sl].astype(np.int16))
                    idxB[2 * (base_t + t) + h] = _wrap_idx(Jpad[sl].astype(np.int16))
        unperm.append(pad_pos)

        HTs = np.ascontiguousarray(
            Hp[:, c * NP:(c + 1) * NP, :].transpose(0, 2, 1)
        )                                                          # (B, C, NP)
        in_maps.append({
            "HT": HTs, "W1": W1, "B1": B1, "W2": W2, "B2": B2,
            "GBD": GBD, "IDXA": idxA, "IDXB": idxB,
        })

    import os
    import tempfile
    trace = bool(os.environ.get("BASS_KERNEL_TRACE"))
    tdir = None
    if trace:
        base = "/root/problem/work"
        tdir = tempfile.mkdtemp(prefix="ktrace_", dir=base if os.path.isdir(base) else None)
    res = run_bass_kernel_spmd(
        nc, in_maps, list(range(NCORES)), trace=trace, tmpdir=tdir,
    )
    if trace:
        kernel.last_trace_dir = tdir
    if trace:
        kernel.last_exec_time_ns = res.exec_time_ns

    logits = np.empty((B, L, E), np.float32)
    for c in range(NCORES):
        dev = res.results[c]["OUT"]                                # (96, nT2*512)
        # partition p = 24*gg + 12*b + l ; col = 512*T + j ;
        # padded pos = 2048*T + 512*gg + j
        # partition p = 24*k + 12*b + l ; col = 512*T + 128*g + e ;
        # padded pos = 2048*T + 512*g + 128*k + e
        dv = dev.reshape(4, 2, L, nT2, 4, 128)                     # (k, b, l, T, g, e)
        dv = dv.transpose(1, 2, 3, 4, 0, 5).reshape(B, L, Epad)    # (b, l, padded)
        pad_pos = unperm[c]
        valid = pad_pos >= 0
        logits[:, :, c * ESHARD + pad_pos[valid]] = dv[:, :, valid]
    return logits


# revision 15
# speedup vs baseline: 1.1317x; 1.1073x over previous
"""DirectedLowRankEdgeScorer TRN2 Bass kernel (8 NeuronCores, SPMD).

logits[b,l,e] = sum_r a[b,I[e],r] * gamma[l,r] * b[b,J[e],r]
  a = relu(H@W1s+b1s)@W2s+b2s,  b = relu(H@W1d+b1d)@W2d+b2d,  H = X[:,-1]

Per-core plan:
  1. MLP (true fp32 on PE) over this core's 1/8 node shard -> 256B records
     rec[n] = [a0 a1 b0 b1] (4x16 fp32), written to DRAM.
  2. AllGather record shards -> full record table in every core's HBM.
  3. Edge shard (E/8) gathered via dma_gather (4 SWDGE queues, 256B/desc,
     int16 indices; host pre-groups edges by (I>=32768, J>=32768) so each
     1024-gather uses a single table base).
  4. DVE: prod = aI * bJ (both batches, 32 wide).  PE: transpose 128x32 ->
     32x128, then block-diag gamma matmul in 3x-bf16 (hi/lo split, fp32
     accurate) -> psum [96, 512] = (4 subtiles x 2 batches x 12 layers).
  5. Stream [96, 512] tiles to DRAM; host un-permutes to (B, L, E).
"""

import sys
import types

import numpy as np
import ml_dtypes

import bass_rust
import concourse.bass as bass
import concourse.bacc as bacc
import concourse.mybir as mybir
from concourse.bass_utils import run_bass_kernel_spmd
from concourse.tile import TileContext
from concourse.vector_clock import ScopedClock
from concourse.masks import make_identity
from concourse.tile import add_dep_helper

BF16 = ml_dtypes.bfloat16

B, T, N, C = 2, 8, 50000, 64
HID, R, L, E = 128, 16, 12, 1600000
NCORES = 8
ESHARD = E // NCORES          # 200000 edges per core
NP = 6272                     # nodes per core shard (49*128)
NPAD = NP * NCORES            # 50176 padded node count
CHUNK = 32768                 # int16 index reach (unused; kept for reference)
H1N, H2N = 3200, 3072         # half-shard split (per-rank rows in rec_h1/rec_h2)
TG = 2048                     # edges per tile-group (2x1024 gathers/side)
KG = 1024                     # indices per dma_gather


# ---------------------------------------------------------------- patches
def _patched_drain_and_barrier(self, tick_clock, wait_clock):
    nc = self.nc
    probe = nc.sync.drain()
    wait_clock.add_sem_waits(probe.ins, ScopedClock({None: tick_clock.global_clock}))
    si = probe.ins.sync_info
    waits = list(si.on_wait) if si is not None else []
    if len(waits) > 1:
        si.on_wait.clear()
        si.on_wait.append(waits[0])
        for w in waits[1:]:
            ni = nc.sync.drain().ins
            ni.sync_info = bass_rust.SyncInfo(on_wait=[w], on_update=[])
    nc.all_engine_barrier()
    assert self.sems is not None
    popped = nc._tile_sem_poison_stack.pop()
    assert popped is self._sem_poison
    nc.clear_and_free_semaphores(list(self.sems.allocated().values()))
    nc.all_engine_barrier()


TileContext._drain_and_barrier = _patched_drain_and_barrier

if "antenv.axon_hooks" not in sys.modules:
    _mod = types.ModuleType("antenv.axon_hooks")
    _state = {"hook": None}
    _mod.set_axon_ntff_profile_hook = lambda h: _state.__setitem__("hook", h)
    _mod.get_axon_ntff_profile_hook = lambda: _state["hook"]
    sys.modules["antenv.axon_hooks"] = _mod
    try:
        import antenv

        antenv.axon_hooks = _mod
    except Exception:
        pass
    try:
        from trn_agent_boot.trn_boot import _ntff_profile_via_ctypes

        _hook = _ntff_profile_via_ctypes("/opt/axon/libaxon_pjrt.so")
        if _hook is not None:
            _mod.set_axon_ntff_profile_hook(_hook)
    except Exception:
        pass


# ---------------------------------------------------------------- device
_PROGRAM_CACHE = {}


def build_program(tiles_per_group):
    """tiles_per_group: 4 ints, 2048-edge tile-group count per (cI, cJ) group."""
    nT2 = sum(tiles_per_group)
    nT = 2 * nT2
    f32, bf16, i16 = mybir.dt.float32, mybir.dt.bfloat16, mybir.dt.int16

    nc = bacc.Bacc("TRN2", target_bir_lowering=False, num_swdge_queues=4)

    HT = nc.declare_dram_parameter("HT", [B, C, NP], f32, isOutput=False)
    W1 = nc.declare_dram_parameter("W1", [2, C, HID], f32, isOutput=False)
    B1 = nc.declare_dram_parameter("B1", [2, HID, 1], f32, isOutput=False)
    W2 = nc.declare_dram_parameter("W2", [2, HID, R], f32, isOutput=False)
    B2 = nc.declare_dram_parameter("B2", [2, 128, R], f32, isOutput=False)
    GBD = nc.declare_dram_parameter("GBD", [2, 128, 96], bf16, isOutput=False)
    IDXA = nc.declare_dram_parameter("IDXA", [nT, 128, KG // 16], i16, isOutput=False)
    IDXB = nc.declare_dram_parameter("IDXB", [nT, 128, KG // 16], i16, isOutput=False)
    OUT = nc.declare_dram_parameter("OUT", [96, nT2 * 512], f32, isOutput=True)

    rec_shard = nc.dram_tensor("rec_shard", [NP, 64], f32)
    H1N, H2N = 3200, 3072
    rec_h1 = nc.dram_tensor("rec_h1", [NCORES * H1N, 64], f32, addr_space="Shared")
    rec_h2 = nc.dram_tensor("rec_h2", [NCORES * H2N, 64], f32, addr_space="Shared")

    group_of_tile = []
    for g in range(4):
        group_of_tile += [g] * tiles_per_group[g]

    with TileContext(nc) as tc:
        with (
            tc.tile_pool(name="const", bufs=1) as constp,
            tc.tile_pool(name="h1p", bufs=1) as h1p,
            tc.tile_pool(name="recp", bufs=3) as recp,
            tc.tile_pool(name="gp", bufs=3) as gp,
            tc.tile_pool(name="prodp", bufs=3) as prodp,
            tc.tile_pool(name="ctp", bufs=3) as ctp,
            tc.tile_pool(name="outp", bufs=3) as outp,
            tc.tile_pool(name="ps1", bufs=2, space="PSUM") as ps1,
            tc.tile_pool(name="ps2", bufs=2, space="PSUM") as ps2,
            tc.tile_pool(name="psT", bufs=2, space="PSUM") as psT,
            tc.tile_pool(name="psL", bufs=2, space="PSUM") as psL,
        ):
            w1_s = constp.tile([C, 2, HID], f32)
            nc.sync.dma_start(w1_s[:], W1[:].rearrange("t c h -> c t h"))
            b1_s = constp.tile([HID, 2, 1], f32)
            nc.sync.dma_start(b1_s[:], B1[:].rearrange("t h o -> h t o"))
            w2_s = constp.tile([HID, 2, R], f32)
            nc.sync.dma_start(w2_s[:], W2[:].rearrange("t h r -> h t r"))
            b2_s = constp.tile([128, 2, R], f32)
            nc.sync.dma_start(b2_s[:], B2[:].rearrange("t p r -> p t r"))
            ht_s = constp.tile([C, B, NP], f32)
            nc.sync.dma_start(ht_s[:], HT[:].rearrange("b c n -> c b n"))
            idxa_all = constp.tile([128, nT, KG // 16], i16)
            nc.sync.dma_start(idxa_all[:], IDXA[:].rearrange("t p x -> p t x"))
            idxb_all = constp.tile([128, nT, KG // 16], i16)
            nc.sync.dma_start(idxb_all[:], IDXB[:].rearrange("t p x -> p t x"))
            gbd_s = constp.tile([128, 2, 96], bf16)
            nc.sync.dma_start(gbd_s[:], GBD[:].rearrange("t p m -> p t m"))
            ident = constp.tile([128, 128], f32)
            make_identity(nc, ident[:])

            # ---- MLP passes; each pass ends with its half-shard AllGather
            cc_insts = []
            for (p0, psz) in ((0, H1N), (H1N, H2N)):
                h1t = {}
                for t in range(2):
                    for b in range(B):
                        h1x = h1p.tile([HID, H1N], f32, tag=f"h1_{t}_{b}")
                        h1t[(t, b)] = h1x
                for n0 in range(0, psz, 512):
                    csz = min(512, psz - n0)
                    for t in range(2):
                        for b in range(B):
                            p1 = ps1.tile([HID, csz], f32, tag="p1")
                            nc.tensor.matmul(
                                p1[:],
                                w1_s[:, t, :],
                                ht_s[:, b, p0 + n0:p0 + n0 + csz],
                            )
                            nc.scalar.activation(
                                h1t[(t, b)][:, n0:n0 + csz], p1[:],
                                mybir.ActivationFunctionType.Relu,
                                bias=b1_s[:, t, :], scale=1.0,
                            )
                rec_dmas = []
                for s in range(psz // 128):
                    rec = recp.tile([128, 64], f32, tag="rec")
                    for t in range(2):
                        for b in range(B):
                            p2 = ps2.tile([128, R], f32, tag="p2")
                            nc.tensor.matmul(
                                p2[:],
                                h1t[(t, b)][:, s * 128:(s + 1) * 128],
                                w2_s[:, t, :],
                            )
                            co = 32 * t + 16 * b
                            nc.vector.tensor_add(
                                rec[:, co:co + 16], p2[:], b2_s[:, t, :]
                            )
                    n0 = p0 + s * 128
                    di = nc.sync.dma_start(rec_shard[n0:n0 + 128, :], rec[:])
                    rec_dmas.append(di)
                dst = rec_h1 if p0 == 0 else rec_h2
                cc = nc.gpsimd.collective_compute(
                    "AllGather",
                    mybir.AluOpType.bypass,
                    replica_groups=[list(range(NCORES))],
                    ins=[rec_shard[p0:p0 + psz, :]],
                    outs=[dst[:]],
                )
                for di in rec_dmas:
                    add_dep_helper(cc.ins, di.ins, True, "cc waits rec dmas")
                if cc_insts:
                    add_dep_helper(cc.ins, cc_insts[-1].ins, True, "cc order")
                cc_insts.append(cc)

            # ---- gather + score
            for Ti in range(nT2):
                g = group_of_tile[Ti]
                recA = rec_h1 if g < 2 else rec_h2
                recB = rec_h1 if g % 2 == 0 else rec_h2
                ccA = cc_insts[0] if g < 2 else cc_insts[1]
                ccB = cc_insts[0] if g % 2 == 0 else cc_insts[1]

                prods = []
                for h in range(2):
                    gA = gp.tile([128, 8, 64], f32, tag=f"gA{h}")
                    gB = gp.tile([128, 8, 64], f32, tag=f"gB{h}")
                    ga_i = nc.gpsimd.dma_gather(
                        gA[:],
                        recA[:],
                        idxa_all[:, 2 * Ti + h, :],
                        num_idxs=KG, num_idxs_reg=KG, elem_size=64,
                        single_packet=False,
                        queue_num=(2 * Ti + h) % 4,
                    )
                    add_dep_helper(ga_i.ins, ccA.ins, True, "gather waits cc")
                    gb_i = nc.gpsimd.dma_gather(
                        gB[:],
                        recB[:],
                        idxb_all[:, 2 * Ti + h, :],
                        num_idxs=KG, num_idxs_reg=KG, elem_size=64,
                        single_packet=False,
                        queue_num=(2 * Ti + h + 2) % 4,
                    )
                    add_dep_helper(gb_i.ins, ccB.ins, True, "gather waits cc")
                    prodh = prodp.tile([128, 8, 32], f32, tag=f"prod{h}")
                    nc.vector.tensor_mul(prodh[:], gA[:, :, 0:32], gB[:, :, 32:64])
                    prods.append(prodh)

                pT = psT.tile([128, 512], f32, tag="pT")
                for gg in range(4):
                    nc.tensor.transpose(
                        pT[:, 128 * gg:128 * (gg + 1)],
                        prods[gg // 2][:, 4 * (gg % 2):4 * (gg % 2 + 1), :],
                        ident[:],
                    )
                cHi = ctp.tile([128, 512], bf16, tag="cHi")
                nc.scalar.copy(cHi[:], pT[:])
                cLo = ctp.tile([128, 512], bf16, tag="cLo")
                nc.vector.tensor_sub(cLo[:], pT[:], cHi[:])

                pL = psL.tile([96, 512], f32, tag="pL")
                nc.tensor.matmul(pL[:], gbd_s[:, 0, :], cHi[:], start=True, stop=False)
                nc.tensor.matmul(pL[:], gbd_s[:, 0, :], cLo[:], start=False, stop=False)
                nc.tensor.matmul(pL[:], gbd_s[:, 1, :], cHi[:], start=False, stop=True)

                outS = outp.tile([96, 512], f32, tag="outS")
                nc.scalar.copy(outS[:], pL[:])
                nc.sync.dma_start(OUT[:, 512 * Ti:512 * (Ti + 1)], outS[:])

    nc.finalize()
    return nc


# ---------------------------------------------------------------- host
def _wrap_idx(flat_idx):
    """[KG] int16 -> [128, KG//16] wrapped-16, replicated x8."""
    w = flat_idx.reshape(KG // 16, 16).T
    return np.tile(w, (8, 1))


def kernel(X, edge_index, W1s, b1s, W2s, b2s, W1d, b1d, W2d, b2d, gamma):
    X = np.asarray(X)
    edge_index = np.asarray(edge_index)
    H = np.ascontiguousarray(X[:, -1]).astype(np.float32)          # (B, N, C)
    Hp = np.zeros((B, NPAD, C), np.float32)
    Hp[:, :N] = H

    I = edge_index[0].astype(np.int64)
    J = edge_index[1].astype(np.int64)

    # per-core edge groups: key by which half-table (rec_h1/rec_h2) each
    # endpoint lives in; rows are rank-concatenated half-shards.
    def _rowbuf(nodes):
        r = nodes // NP
        i = nodes % NP
        in1 = i < H1N
        row = np.where(in1, H1N * r + i, H2N * r + (i - H1N))
        return row, in1

    core_data = []
    counts = np.zeros((NCORES, 4), np.int64)
    for c in range(NCORES):
        Ic = I[c * ESHARD:(c + 1) * ESHARD]
        Jc = J[c * ESHARD:(c + 1) * ESHARD]
        rI, b1I = _rowbuf(Ic)
        rJ, b1J = _rowbuf(Jc)
        gk = np.where(b1I, 0, 2) + np.where(b1J, 0, 1)
        order = np.argsort(gk, kind="stable")
        core_data.append((rI[order], rJ[order], gk[order], order))
        for g in range(4):
            counts[c, g] = int((gk == g).sum())

    tiles_per_group = tuple(
        int(-(-counts[:, g].max() // TG)) for g in range(4)
    )
    nT2 = sum(tiles_per_group)
    nT = 2 * nT2
    Epad = nT2 * TG

    key = tiles_per_group
    if key not in _PROGRAM_CACHE:
        _PROGRAM_CACHE[key] = build_program(tiles_per_group)
    nc = _PROGRAM_CACHE[key]

    # group -> padded tile offsets
    goff = np.zeros(5, np.int64)
    for g in range(4):
        goff[g + 1] = goff[g] + tiles_per_group[g]

    # shared weight tensors
    W1 = np.stack([W1s, W1d]).astype(np.float32)                  # (2, C, HID)
    B1 = np.stack([b1s, b1d]).astype(np.float32)[:, :, None]      # (2, HID, 1)
    W2 = np.stack([W2s, W2d]).astype(np.float32)                  # (2, HID, R)
    B2 = np.stack(
        [np.tile(b2s[None, :], (128, 1)), np.tile(b2d[None, :], (128, 1))]
    ).astype(np.float32)                                          # (2, 128, R)

    gbd = np.zeros((128, 96), np.float32)
    gT = np.asarray(gamma, np.float32).T                          # (R, L)
    for g in range(4):
        for b in range(B):
            gbd[32 * g + 16 * b:32 * g + 16 * b + 16,
                24 * g + 12 * b:24 * g + 12 * b + 12] = gT
    gbd_hi = gbd.astype(BF16)
    gbd_lo = (gbd - gbd_hi.astype(np.float32)).astype(BF16)
    GBD = np.stack([gbd_hi, gbd_lo])                              # (2, 128, 96)

    in_maps = []
    unperm = []  # per core: padded position -> original in-shard position (or -1)
    for c in range(NCORES):
        Ic, Jc, gk, order = core_data[c]
        idxA = np.zeros((nT, 128, KG // 16), np.int16)
        idxB = np.zeros((nT, 128, KG // 16), np.int16)
        pad_pos = np.full(Epad, -1, np.int64)
        rp = 0  # read ptr into sorted arrays
        for g in range(4):
            cnt = int(counts[c, g])
            base_t = int(goff[g])
            Ig = Ic[rp:rp + cnt]
            Jg = Jc[rp:rp + cnt]
            og = order[rp:rp + cnt]
            rp += cnt
            ntile_g = tiles_per_group[g]
            Ipad = np.zeros(ntile_g * TG, np.int64)
            Jpad = np.zeros(ntile_g * TG, np.int64)
            Ipad[:cnt] = Ig
            Jpad[:cnt] = Jg
            pad_pos[base_t * TG: base_t * TG + cnt] = og
            for t in range(ntile_g):
                for h in range(2):
                    sl = slice(t * TG + h * KG, t * TG + (h + 1) * KG)
                    idxA[2 * (base_t + t) + h] = _wrap_idx(Ipad[sl].astype(np.int16))
                    idxB[2 * (base_t + t) + h] = _wrap_idx(Jpad[sl].astype(np.int16))
        unperm.append(pad_pos)

        HTs = np.ascontiguousarray(
            Hp[:, c * NP:(c + 1) * NP, :].transpose(0, 2, 1)
        )                                                          # (B, C, NP)
        in_maps.append({
            "HT": HTs, "W1": W1, "B1": B1, "W2": W2, "B2": B2,
            "GBD": GBD, "IDXA": idxA, "IDXB": idxB,
        })

    import os
    import tempfile
    trace = bool(os.environ.get("BASS_KERNEL_TRACE"))
    tdir = None
    if trace:
        base = "/root/problem/work"
        tdir = tempfile.mkdtemp(prefix="ktrace_", dir=base if os.path.isdir(base) else None)
    res = run_bass_kernel_spmd(
        nc, in_maps, list(range(NCORES)), trace=trace, tmpdir=tdir,
    )
    if trace:
        kernel.last_trace_dir = tdir
    if trace:
        kernel.last_exec_time_ns = res.exec_time_ns

    logits = np.empty((B, L, E), np.float32)
    for c in range(NCORES):
        dev = res.results[c]["OUT"]                                # (96, nT2*512)
        # partition p = 24*gg + 12*b + l ; col = 512*T + j ;
        # padded pos = 2048*T + 512*gg + j
        # partition p = 24*k + 12*b + l ; col = 512*T + 128*g + e ;
        # padded pos = 2048*T + 512*g + 128*k + e
        dv = dev.reshape(4, 2, L, nT2, 4, 128)                     # (k, b, l, T, g, e)
        dv = dv.transpose(1, 2, 3, 4, 0, 5).reshape(B, L, Epad)    # (b, l, padded)
        pad_pos = unperm[c]
        valid = pad_pos >= 0
        logits[:, :, c * ESHARD + pad_pos[valid]] = dv[:, :, valid]
    return logits
